# revision 7
# baseline (speedup 1.0000x reference)

# Causal self-attention (B=8, T=1024, C=768, H=12) on 8 trn2 NeuronCores.
# Strategy: pure data parallelism — one batch element per core. Each core runs
# a fused QKV -> causal attention -> c_proj kernel written in Bass/Tile.
#
# Device-side layout choices (per core, T=1024, C=768, H=12, D=64):
#   - host pre-transposes x to xT [C, T] so every matmul contraction dim lands
#     on SBUF partitions; no on-device transposes anywhere.
#   - q,k are produced transposed (qT/kT [D,T] per head, 2 heads per 128-row
#     SBUF tile); v is produced in natural [T, C] layout.
#   - scoresT[tk, tq] = kT.T @ qT (contraction over D=64; two heads packed in
#     the PE array via row groups 0/64).
#   - softmax without max-subtraction (logits are bounded, exp is safe in f32);
#     exp on ScalarE over a 2-head [128, 1024] PSUM tile in one instruction;
#     causal masking applied post-exp on SBUF (triangle multiply + block
#     zeroing only on diagonal tiles).
#   - denominators come from an extra ones-matmul (column sums via PE),
#     replicated across 64 partitions so the reciprocal broadcast is free.
#   - PV matmul accumulates yT[c, t] directly in the layout c_proj needs.
#   - v-bias and proj-bias are folded on the host:
#       y = Pnorm @ (v~ + 1 b_v) = Pnorm @ v~ + 1 b_v
#       out = y @ w_proj + b_proj = (Pnorm @ v~) @ w_proj + (b_v @ w_proj + b_proj)
#     so the device never adds b_v; b_total is added once after c_proj.
#   - the 1/sqrt(D) scale and b_q are folded into w_attn's q columns (exact:
#     division by 8 is exact in fp32).
#   - all matmuls run as float32r (full PE rate at moving-dim >= 256).

import sys

sys.path.insert(0, "/opt/trn_rl_repo")

import numpy as np

import concourse.bass as bass
import concourse.bacc as bacc
import concourse.mybir as mybir
import concourse.tile as tile
from concourse.vector_clock import ScopedClock

B, T, C, H = 8, 1024, 768, 12
D = C // H  # 64
NCORES = 8
F32 = mybir.dt.float32
F32R = mybir.dt.float32r
BF16 = mybir.dt.bfloat16

# ---------------------------------------------------------------------------
# This walrus build accepts only one sync wait per CTRL instruction; the Tile
# tail drain aggregates one wait per logical processor. Split the excess waits
# onto dedicated NOPs ahead of the drain.
_MAX_WAITS = 1
_PATCHED = False


def _patch_tile_drain():
    global _PATCHED
    if _PATCHED:
        return
    _PATCHED = True

    def _drain_and_barrier(self, tick_clock, wait_clock):
        nc = self.nc
        bb = nc.cur_bb.bb
        idx_before = len(bb.instructions)
        drain_inst = nc.sync.drain()
        wait_clock.add_sem_waits(
            drain_inst.ins, ScopedClock({None: tick_clock.global_clock})
        )
        si = drain_inst.ins.sync_info
        if si is not None and si.on_wait and len(si.on_wait) > _MAX_WAITS:
            waits = list(si.on_wait)
            si.on_wait = waits[:_MAX_WAITS]
            extra = waits[_MAX_WAITS:]
            nops = []
            for i in range(0, len(extra), _MAX_WAITS):
                nop = nc.sync.nop(hint=f"drain_wait_spill_{i}", nofuse=True)
                nop.ins.sync_info = mybir.SyncInfo(
                    on_wait=extra[i : i + _MAX_WAITS], on_update=[]
                )
                nops.append(nop.ins)
            insts = bb.instructions
            tail = list(insts[idx_before:])
            assert tail[0] is drain_inst.ins
            del insts[idx_before:]
            for n in nops:
                insts.append(n)
            for t in tail:
                insts.append(t)
        nc.all_engine_barrier()
        popped = nc._tile_sem_poison_stack.pop()
        assert popped is self._sem_poison
        nc.clear_and_free_semaphores(list(self.sems.allocated().values()))
        nc.all_engine_barrier()

    tile.TileContext._drain_and_barrier = _drain_and_barrier


# ---------------------------------------------------------------------------


def build_program():
    _patch_tile_drain()
    nc = bacc.Bacc("TRN2", target_bir_lowering=False, debug=False)

    xT = nc.dram_tensor("xT", [C, T], F32R, kind="ExternalInput").ap()
    wA = nc.dram_tensor("w_attn", [C, 3 * C], F32R, kind="ExternalInput").ap()
    wP = nc.dram_tensor("w_proj", [C, C], F32R, kind="ExternalInput").ap()
    bqk = nc.dram_tensor("bqk", [128, 12], F32, kind="ExternalInput").ap()
    btot = nc.dram_tensor("btot", [1, C], F32, kind="ExternalInput").ap()
    tri2 = nc.dram_tensor("tri2", [128, 2, 128], BF16, kind="ExternalInput").ap()
    ones64 = nc.dram_tensor("ones64", [128, D], BF16, kind="ExternalInput").ap()
    out = nc.dram_tensor("out", [T, C], F32, kind="ExternalOutput").ap()

    EXP = mybir.ActivationFunctionType.Exp

    with tile.TileContext(nc) as tc:
        with (
            tc.tile_pool(name="persist", bufs=1) as persist,
            tc.tile_pool(name="wqk", bufs=24) as wqk_pool,
            tc.tile_pool(name="qk", bufs=4) as qk_pool,
            tc.tile_pool(name="ex", bufs=4) as ex_pool,
            tc.tile_pool(name="rb", bufs=2) as rb_pool,
            tc.tile_pool(name="ob", bufs=2) as ob_pool,
            tc.tile_pool(name="big_ps", bufs=2, space="PSUM") as big_ps,
            tc.tile_pool(name="acc_ps", bufs=2, space="PSUM") as acc_ps,
        ):
            # ---- persistent SBUF tensors -------------------------------
            xT_sb = []
            for c in range(6):
                t_ = persist.tile([128, T], F32R, tag=f"xT{c}")
                nc.sync.dma_start(out=t_[:], in_=xT[128 * c : 128 * (c + 1), :])
                xT_sb.append(t_)
            wv_sb = []
            for c in range(6):
                t_ = persist.tile([128, C], F32R, tag=f"wv{c}")
                nc.sync.dma_start(
                    out=t_[:], in_=wA[128 * c : 128 * (c + 1), 2 * C : 3 * C]
                )
                wv_sb.append(t_)
            wp_sb = []
            for c in range(6):
                t_ = persist.tile([128, C], F32R, tag=f"wp{c}")
                nc.sync.dma_start(out=t_[:], in_=wP[128 * c : 128 * (c + 1), :])
                wp_sb.append(t_)
            bqk_sb = persist.tile([128, 12], F32, tag="bqk")
            nc.sync.dma_start(out=bqk_sb[:], in_=bqk[:, :])
            tri_sb = persist.tile([128, 2, 128], BF16, tag="tri")
            nc.sync.dma_start(out=tri_sb[:], in_=tri2[:, :, :])
            ones_sb = persist.tile([128, D], BF16, tag="ones")
            nc.sync.dma_start(out=ones_sb[:], in_=ones64[:, :])
            btot_sb = persist.tile([128, C], F32, tag="btot")
            btot_bcast = bass.AP(
                tensor=btot.tensor, offset=btot.offset, ap=[[0, 128], [1, C]]
            )
            nc.sync.dma_start(out=btot_sb[:], in_=btot_bcast)
            v_sb = [
                persist.tile([128, C], BF16, tag=f"v{t}", name=f"v{t}")
                for t in range(8)
            ]
            yT_sb = [
                persist.tile([128, T], F32R, tag=f"yT{p}", name=f"yT{p}")
                for p in range(6)
            ]

            # ---- V projection: v[t, c] natural layout ------------------
            for t in range(8):
                vps = big_ps.tile([128, C], F32, tag="scores")
                for c in range(6):
                    for n0, nw in ((0, 512), (512, 256)):
                        nc.tensor.matmul(
                            vps[:, n0 : n0 + nw],
                            lhsT=xT_sb[c][:, 128 * t : 128 * (t + 1)],
                            rhs=wv_sb[c][:, n0 : n0 + nw],
                            start=(c == 0),
                            stop=(c == 5),
                        )
                nc.vector.tensor_copy(v_sb[t][:], vps[:, :])

            # ---- per head-pair: q/k projection + attention -------------
            for p in range(6):
                qk_tiles = {}
                for kind, jt in (("q", p), ("k", 6 + p)):
                    qk = qk_pool.tile([128, T], F32R, tag=f"qk_{kind}")
                    for th in range(2):
                        ps = big_ps.tile([128, 512], F32, tag="scores")
                        for c in range(6):
                            w_t = wqk_pool.tile([128, 128], F32R, tag="wqk")
                            nc.sync.dma_start(
                                out=w_t[:],
                                in_=wA[
                                    128 * c : 128 * (c + 1), 128 * jt : 128 * (jt + 1)
                                ],
                            )
                            nc.tensor.matmul(
                                ps[:, :],
                                lhsT=w_t[:],
                                rhs=xT_sb[c][:, 512 * th : 512 * (th + 1)].bitcast(
                                    F32R
                                ),
                                start=(c == 0),
                                stop=(c == 5),
                            )
                        nc.vector.tensor_scalar_add(
                            qk[:, 512 * th : 512 * (th + 1)],
                            ps[:, :],
                            bqk_sb[:, jt : jt + 1],
                        )
                    qk_tiles[kind] = qk
                qT, kT = qk_tiles["q"], qk_tiles["k"]

                for i in range(2):
                    pv = acc_ps.tile([128, 512], F32, tag="pv")
                    sm = acc_ps.tile([128, 512], F32, tag="sums")
                    njt = 4 * i + 4  # causal tk tiles for this tq half
                    for j in range(njt):
                        sc = big_ps.tile([128, 1024], F32, tag="scores")
                        for h in range(2):
                            nc.tensor.matmul(
                                sc[:, 512 * h : 512 * (h + 1)],
                                lhsT=kT[64 * h : 64 * (h + 1), 128 * j : 128 * (j + 1)]
                                ,
                                rhs=qT[64 * h : 64 * (h + 1), 512 * i : 512 * (i + 1)]
                                ,
                                tile_position=(64 * h, 0),
                            )
                        ex = ex_pool.tile([128, 1024], BF16, tag="ex")
                        nc.scalar.activation(ex[:, :], sc[:, :], EXP)
                        r = j - 4 * i
                        if r >= 0:
                            ex3 = ex[:].rearrange("p (h c) -> p h c", h=2)
                            if r > 0:
                                nc.vector.memset(ex3[:, :, 0 : 128 * r], 0.0)
                            nc.vector.tensor_mul(
                                ex3[:, :, 128 * r : 128 * (r + 1)],
                                ex3[:, :, 128 * r : 128 * (r + 1)],
                                tri_sb[:, :, :],
                            )
                        first, last = (j == 0), (j == njt - 1)
                        for h in range(2):
                            hd = 2 * p + h
                            nc.tensor.matmul(
                                pv[64 * h : 64 * (h + 1), :],
                                lhsT=v_sb[j][:, D * hd : D * (hd + 1)],
                                rhs=ex[:, 512 * h : 512 * (h + 1)],
                                tile_position=(0, 64 * h),
                                start=first,
                                stop=last,
                            )
                            nc.tensor.matmul(
                                sm[64 * h : 64 * (h + 1), :],
                                lhsT=ones_sb[:],
                                rhs=ex[:, 512 * h : 512 * (h + 1)],
                                tile_position=(0, 64 * h),
                                start=first,
                                stop=last,
                            )
                    rb = rb_pool.tile([128, 512], F32, tag="rb")
                    nc.vector.reciprocal_approx_fast(rb[:], sm[:, :])
                    nc.vector.tensor_mul(
                        yT_sb[p][:, 512 * i : 512 * (i + 1)], pv[:, :], rb[:]
                    )

            # ---- c_proj ------------------------------------------------
            for t in range(8):
                cps = big_ps.tile([128, C], F32, tag="scores")
                for c in range(6):
                    for n0, nw in ((0, 512), (512, 256)):
                        nc.tensor.matmul(
                            cps[:, n0 : n0 + nw],
                            lhsT=yT_sb[c][:, 128 * t : 128 * (t + 1)],
                            rhs=wp_sb[c][:, n0 : n0 + nw],
                            start=(c == 0),
                            stop=(c == 5),
                        )
                ob = ob_pool.tile([128, C], F32, tag="ob")
                nc.vector.tensor_add(ob[:], cps[:, :], btot_sb[:])
                nc.sync.dma_start(out=out[128 * t : 128 * (t + 1), :], in_=ob[:])

    nc.compile()
    return nc


_NC = None


def _get_nc():
    global _NC
    if _NC is None:
        _NC = build_program()
    return _NC


def round_fp32r(a):
    """Round fp32 to fp32r (e8m11): round-to-nearest-even at mantissa bit 12.
    Matches walrus's fp32_to_fp32r (downconv to e8m11, stored in the high 20
    bits, i.e. fp32 with the low 12 mantissa bits zero)."""
    u = np.ascontiguousarray(a, dtype=np.float32).view(np.uint32)
    r = (u + np.uint32(0x7FF) + ((u >> np.uint32(12)) & np.uint32(1))) & np.uint32(
        0xFFFFF000
    )
    return r.view(np.float32)


def make_inputs(x, w_attn, b_attn, w_proj, b_proj):
    """Host-side prep: fold scales/biases, transpose x, build constants."""
    x = np.asarray(x, dtype=np.float32)
    w_attn = np.asarray(w_attn, dtype=np.float32)
    b_attn = np.asarray(b_attn, dtype=np.float32)
    w_proj = np.asarray(w_proj, dtype=np.float32)
    b_proj = np.asarray(b_proj, dtype=np.float32)

    wA = w_attn.copy()
    wA[:, :C] *= 0.125  # fold 1/sqrt(D)=1/8 into q columns (exact in fp32)
    bq = b_attn[:C] * 0.125
    bk = b_attn[C : 2 * C]
    bv = b_attn[2 * C :]
    # bqk[p, j] = bias for feature j*128+p, j in 0..11 (q tiles then k tiles)
    bqk = np.concatenate([bq, bk]).reshape(12, 128).T.copy()
    btot = (b_proj.astype(np.float64) + bv.astype(np.float64) @ w_proj.astype(np.float64)).astype(
        np.float32
    )[None, :]
    import ml_dtypes
    tri = np.triu(np.ones((128, 128), dtype=np.float32))
    tri2 = np.stack([tri, tri], axis=1).astype(ml_dtypes.bfloat16)  # [128, 2, 128]
    ones64 = np.ones((128, D), dtype=ml_dtypes.bfloat16)

    shared = {
        "w_attn": round_fp32r(wA),
        "w_proj": round_fp32r(w_proj),
        "bqk": np.ascontiguousarray(bqk),
        "btot": np.ascontiguousarray(btot),
        "tri2": np.ascontiguousarray(tri2),
        "ones64": ones64,
    }
    in_maps = []
    for b in range(B):
        m = dict(shared)
        m["xT"] = round_fp32r(x[b].T)
        in_maps.append(m)
    return in_maps


def kernel(x, w_attn, b_attn, w_proj, b_proj):
    from concourse.bass_utils import run_bass_kernel_spmd

    nc = _get_nc()
    in_maps = make_inputs(x, w_attn, b_attn, w_proj, b_proj)
    res = run_bass_kernel_spmd(nc, in_maps, list(range(NCORES)))
    out = np.stack([res.results[b]["out"] for b in range(B)], axis=0)
    return out.astype(np.float32)


# revision 8
# speedup vs baseline: 4622.6460x; 4622.6460x over previous

# Causal self-attention (B=8, T=1024, C=768, H=12) on 8 trn2 NeuronCores.
# Strategy: pure data parallelism — one batch element per core. Each core runs
# a fused QKV -> causal attention -> c_proj kernel written in Bass/Tile.
#
# Device-side layout choices (per core, T=1024, C=768, H=12, D=64):
#   - host pre-transposes x to xT [C, T] so every matmul contraction dim lands
#     on SBUF partitions; no on-device transposes anywhere.
#   - q,k are produced transposed (qT/kT [D,T] per head, 2 heads per 128-row
#     SBUF tile); v is produced in natural [T, C] layout.
#   - scoresT[tk, tq] = kT.T @ qT (contraction over D=64; two heads packed in
#     the PE array via row groups 0/64).
#   - softmax without max-subtraction (logits are bounded, exp is safe in f32);
#     exp on ScalarE over a 2-head [128, 1024] PSUM tile in one instruction;
#     causal masking applied post-exp on SBUF (triangle multiply + block
#     zeroing only on diagonal tiles).
#   - denominators come from an extra ones-matmul (column sums via PE),
#     replicated across 64 partitions so the reciprocal broadcast is free.
#   - PV matmul accumulates yT[c, t] directly in the layout c_proj needs.
#   - v-bias and proj-bias are folded on the host:
#       y = Pnorm @ (v~ + 1 b_v) = Pnorm @ v~ + 1 b_v
#       out = y @ w_proj + b_proj = (Pnorm @ v~) @ w_proj + (b_v @ w_proj + b_proj)
#     so the device never adds b_v; b_total is added once after c_proj.
#   - the 1/sqrt(D) scale and b_q are folded into w_attn's q columns (exact:
#     division by 8 is exact in fp32).
#   - all matmuls run as float32r (full PE rate at moving-dim >= 256).

import sys

sys.path.insert(0, "/opt/trn_rl_repo")

import numpy as np

import concourse.bass as bass
import concourse.bacc as bacc
import concourse.mybir as mybir
import concourse.tile as tile
from concourse.vector_clock import ScopedClock

B, T, C, H = 8, 1024, 768, 12
D = C // H  # 64
NCORES = 8
F32 = mybir.dt.float32
F32R = mybir.dt.float32r
BF16 = mybir.dt.bfloat16

# ---------------------------------------------------------------------------
# This walrus build accepts only one sync wait per CTRL instruction; the Tile
# tail drain aggregates one wait per logical processor. Split the excess waits
# onto dedicated NOPs ahead of the drain.
_MAX_WAITS = 1
_PATCHED = False


def _patch_tile_drain():
    global _PATCHED
    if _PATCHED:
        return
    _PATCHED = True

    def _drain_and_barrier(self, tick_clock, wait_clock):
        nc = self.nc
        bb = nc.cur_bb.bb
        idx_before = len(bb.instructions)
        drain_inst = nc.sync.drain()
        wait_clock.add_sem_waits(
            drain_inst.ins, ScopedClock({None: tick_clock.global_clock})
        )
        si = drain_inst.ins.sync_info
        if si is not None and si.on_wait and len(si.on_wait) > _MAX_WAITS:
            waits = list(si.on_wait)
            si.on_wait = waits[:_MAX_WAITS]
            extra = waits[_MAX_WAITS:]
            nops = []
            for i in range(0, len(extra), _MAX_WAITS):
                nop = nc.sync.nop(hint=f"drain_wait_spill_{i}", nofuse=True)
                nop.ins.sync_info = mybir.SyncInfo(
                    on_wait=extra[i : i + _MAX_WAITS], on_update=[]
                )
                nops.append(nop.ins)
            insts = bb.instructions
            tail = list(insts[idx_before:])
            assert tail[0] is drain_inst.ins
            del insts[idx_before:]
            for n in nops:
                insts.append(n)
            for t in tail:
                insts.append(t)
        nc.all_engine_barrier()
        popped = nc._tile_sem_poison_stack.pop()
        assert popped is self._sem_poison
        nc.clear_and_free_semaphores(list(self.sems.allocated().values()))
        nc.all_engine_barrier()

    tile.TileContext._drain_and_barrier = _drain_and_barrier


# ---------------------------------------------------------------------------


def build_program(loop_n=None):
    _patch_tile_drain()
    nc = bacc.Bacc("TRN2", target_bir_lowering=False, debug=False)

    xT = nc.dram_tensor("xT", [C, T], F32R, kind="ExternalInput").ap()
    wA = nc.dram_tensor("w_attn", [C, 3 * C], F32R, kind="ExternalInput").ap()
    wP = nc.dram_tensor("w_proj", [C, C], F32R, kind="ExternalInput").ap()
    bqk = nc.dram_tensor("bqk", [128, 12], F32, kind="ExternalInput").ap()
    btot = nc.dram_tensor("btot", [1, C], F32, kind="ExternalInput").ap()
    tri2 = nc.dram_tensor("tri2", [128, 2, 128], BF16, kind="ExternalInput").ap()
    ones64 = nc.dram_tensor("ones64", [128, D], BF16, kind="ExternalInput").ap()
    out = nc.dram_tensor("out", [T, C], F32, kind="ExternalOutput").ap()

    with tile.TileContext(nc) as tc:
        if loop_n is None:
            _emit_body(nc, tc, xT, wA, wP, bqk, btot, tri2, ones64, out)
        else:
            with tc.For_i(0, loop_n, 1):
                _emit_body(nc, tc, xT, wA, wP, bqk, btot, tri2, ones64, out)
    nc.compile()
    return nc


def _emit_body(nc, tc, xT, wA, wP, bqk, btot, tri2, ones64, out):
    EXP = mybir.ActivationFunctionType.Exp

    if True:
        with (
            tc.tile_pool(name="persist", bufs=1) as persist,
            tc.tile_pool(name="wqk", bufs=24) as wqk_pool,
            tc.tile_pool(name="qk", bufs=4) as qk_pool,
            tc.tile_pool(name="ex", bufs=4) as ex_pool,
            tc.tile_pool(name="rb", bufs=2) as rb_pool,
            tc.tile_pool(name="ob", bufs=2) as ob_pool,
            tc.tile_pool(name="big_ps", bufs=2, space="PSUM") as big_ps,
            tc.tile_pool(name="acc_ps", bufs=2, space="PSUM") as acc_ps,
        ):
            # ---- persistent SBUF tensors -------------------------------
            xT_sb = []
            for c in range(6):
                t_ = persist.tile([128, T], F32R, tag=f"xT{c}")
                nc.sync.dma_start(out=t_[:], in_=xT[128 * c : 128 * (c + 1), :])
                xT_sb.append(t_)
            wv_sb = []
            for c in range(6):
                t_ = persist.tile([128, C], F32R, tag=f"wv{c}")
                nc.sync.dma_start(
                    out=t_[:], in_=wA[128 * c : 128 * (c + 1), 2 * C : 3 * C]
                )
                wv_sb.append(t_)
            wp_sb = []
            for c in range(6):
                t_ = persist.tile([128, C], F32R, tag=f"wp{c}")
                nc.sync.dma_start(out=t_[:], in_=wP[128 * c : 128 * (c + 1), :])
                wp_sb.append(t_)
            bqk_sb = persist.tile([128, 12], F32, tag="bqk")
            nc.sync.dma_start(out=bqk_sb[:], in_=bqk[:, :])
            tri_sb = persist.tile([128, 2, 128], BF16, tag="tri")
            nc.sync.dma_start(out=tri_sb[:], in_=tri2[:, :, :])
            ones_sb = persist.tile([128, D], BF16, tag="ones")
            nc.sync.dma_start(out=ones_sb[:], in_=ones64[:, :])
            btot_sb = persist.tile([128, C], F32, tag="btot")
            btot_bcast = bass.AP(
                tensor=btot.tensor, offset=btot.offset, ap=[[0, 128], [1, C]]
            )
            nc.sync.dma_start(out=btot_sb[:], in_=btot_bcast)
            v_sb = [
                persist.tile([128, C], BF16, tag=f"v{t}", name=f"v{t}")
                for t in range(8)
            ]
            yT_sb = [
                persist.tile([128, T], F32R, tag=f"yT{p}", name=f"yT{p}")
                for p in range(6)
            ]

            # ---- V projection: v[t, c] natural layout ------------------
            for t in range(8):
                vps = big_ps.tile([128, C], F32, tag="scores")
                for c in range(6):
                    for n0, nw in ((0, 512), (512, 256)):
                        nc.tensor.matmul(
                            vps[:, n0 : n0 + nw],
                            lhsT=xT_sb[c][:, 128 * t : 128 * (t + 1)],
                            rhs=wv_sb[c][:, n0 : n0 + nw],
                            start=(c == 0),
                            stop=(c == 5),
                        )
                nc.vector.tensor_copy(v_sb[t][:], vps[:, :])

            # ---- per head-pair: q/k projection + attention -------------
            for p in range(6):
                qk_tiles = {}
                for kind, jt in (("q", p), ("k", 6 + p)):
                    qk = qk_pool.tile([128, T], F32R, tag=f"qk_{kind}")
                    for th in range(2):
                        ps = big_ps.tile([128, 512], F32, tag="scores")
                        for c in range(6):
                            w_t = wqk_pool.tile([128, 128], F32R, tag="wqk")
                            nc.sync.dma_start(
                                out=w_t[:],
                                in_=wA[
                                    128 * c : 128 * (c + 1), 128 * jt : 128 * (jt + 1)
                                ],
                            )
                            nc.tensor.matmul(
                                ps[:, :],
                                lhsT=w_t[:],
                                rhs=xT_sb[c][:, 512 * th : 512 * (th + 1)].bitcast(
                                    F32R
                                ),
                                start=(c == 0),
                                stop=(c == 5),
                            )
                        nc.vector.tensor_scalar_add(
                            qk[:, 512 * th : 512 * (th + 1)],
                            ps[:, :],
                            bqk_sb[:, jt : jt + 1],
                        )
                    qk_tiles[kind] = qk
                qT, kT = qk_tiles["q"], qk_tiles["k"]

                for i in range(2):
                    pv = acc_ps.tile([128, 512], F32, tag="pv")
                    sm = acc_ps.tile([128, 512], F32, tag="sums")
                    njt = 4 * i + 4  # causal tk tiles for this tq half
                    for j in range(njt):
                        sc = big_ps.tile([128, 1024], F32, tag="scores")
                        for h in range(2):
                            nc.tensor.matmul(
                                sc[:, 512 * h : 512 * (h + 1)],
                                lhsT=kT[64 * h : 64 * (h + 1), 128 * j : 128 * (j + 1)]
                                ,
                                rhs=qT[64 * h : 64 * (h + 1), 512 * i : 512 * (i + 1)]
                                ,
                                tile_position=(64 * h, 0),
                            )
                        ex = ex_pool.tile([128, 1024], BF16, tag="ex")
                        nc.scalar.activation(ex[:, :], sc[:, :], EXP)
                        r = j - 4 * i
                        if r >= 0:
                            ex3 = ex[:].rearrange("p (h c) -> p h c", h=2)
                            if r > 0:
                                nc.vector.memset(ex3[:, :, 0 : 128 * r], 0.0)
                            nc.vector.tensor_mul(
                                ex3[:, :, 128 * r : 128 * (r + 1)],
                                ex3[:, :, 128 * r : 128 * (r + 1)],
                                tri_sb[:, :, :],
                            )
                        first, last = (j == 0), (j == njt - 1)
                        for h in range(2):
                            hd = 2 * p + h
                            nc.tensor.matmul(
                                pv[64 * h : 64 * (h + 1), :],
                                lhsT=v_sb[j][:, D * hd : D * (hd + 1)],
                                rhs=ex[:, 512 * h : 512 * (h + 1)],
                                tile_position=(0, 64 * h),
                                start=first,
                                stop=last,
                            )
                            nc.tensor.matmul(
                                sm[64 * h : 64 * (h + 1), :],
                                lhsT=ones_sb[:],
                                rhs=ex[:, 512 * h : 512 * (h + 1)],
                                tile_position=(0, 64 * h),
                                start=first,
                                stop=last,
                            )
                    rb = rb_pool.tile([128, 512], F32, tag="rb")
                    nc.vector.reciprocal_approx_fast(rb[:], sm[:, :])
                    nc.vector.tensor_mul(
                        yT_sb[p][:, 512 * i : 512 * (i + 1)], pv[:, :], rb[:]
                    )

            # ---- c_proj ------------------------------------------------
            for t in range(8):
                cps = big_ps.tile([128, C], F32, tag="scores")
                for c in range(6):
                    for n0, nw in ((0, 512), (512, 256)):
                        nc.tensor.matmul(
                            cps[:, n0 : n0 + nw],
                            lhsT=yT_sb[c][:, 128 * t : 128 * (t + 1)],
                            rhs=wp_sb[c][:, n0 : n0 + nw],
                            start=(c == 0),
                            stop=(c == 5),
                        )
                ob = ob_pool.tile([128, C], F32, tag="ob")
                nc.vector.tensor_add(ob[:], cps[:, :], btot_sb[:])
                nc.sync.dma_start(out=out[128 * t : 128 * (t + 1), :], in_=ob[:])


_NC = None


def _get_nc():
    global _NC
    if _NC is None:
        _NC = build_program()
    return _NC


def round_fp32r(a):
    """Round fp32 to fp32r (e8m11): round-to-nearest-even at mantissa bit 12.
    Matches walrus's fp32_to_fp32r (downconv to e8m11, stored in the high 20
    bits, i.e. fp32 with the low 12 mantissa bits zero)."""
    u = np.ascontiguousarray(a, dtype=np.float32).view(np.uint32)
    r = (u + np.uint32(0x7FF) + ((u >> np.uint32(12)) & np.uint32(1))) & np.uint32(
        0xFFFFF000
    )
    return r.view(np.float32)


def make_inputs(x, w_attn, b_attn, w_proj, b_proj):
    """Host-side prep: fold scales/biases, transpose x, build constants."""
    x = np.asarray(x, dtype=np.float32)
    w_attn = np.asarray(w_attn, dtype=np.float32)
    b_attn = np.asarray(b_attn, dtype=np.float32)
    w_proj = np.asarray(w_proj, dtype=np.float32)
    b_proj = np.asarray(b_proj, dtype=np.float32)

    wA = w_attn.copy()
    wA[:, :C] *= 0.125  # fold 1/sqrt(D)=1/8 into q columns (exact in fp32)
    bq = b_attn[:C] * 0.125
    bk = b_attn[C : 2 * C]
    bv = b_attn[2 * C :]
    # bqk[p, j] = bias for feature j*128+p, j in 0..11 (q tiles then k tiles)
    bqk = np.concatenate([bq, bk]).reshape(12, 128).T.copy()
    btot = (b_proj.astype(np.float64) + bv.astype(np.float64) @ w_proj.astype(np.float64)).astype(
        np.float32
    )[None, :]
    import ml_dtypes
    tri = np.triu(np.ones((128, 128), dtype=np.float32))
    tri2 = np.stack([tri, tri], axis=1).astype(ml_dtypes.bfloat16)  # [128, 2, 128]
    ones64 = np.ones((128, D), dtype=ml_dtypes.bfloat16)

    shared = {
        "w_attn": round_fp32r(wA),
        "w_proj": round_fp32r(w_proj),
        "bqk": np.ascontiguousarray(bqk),
        "btot": np.ascontiguousarray(btot),
        "tri2": np.ascontiguousarray(tri2),
        "ones64": ones64,
    }
    in_maps = []
    for b in range(B):
        m = dict(shared)
        m["xT"] = round_fp32r(x[b].T)
        in_maps.append(m)
    return in_maps


def kernel(x, w_attn, b_attn, w_proj, b_proj):
    from concourse.bass_utils import run_bass_kernel_spmd

    nc = _get_nc()
    in_maps = make_inputs(x, w_attn, b_attn, w_proj, b_proj)
    res = run_bass_kernel_spmd(nc, in_maps, list(range(NCORES)))
    out = np.stack([res.results[b]["out"] for b in range(B)], axis=0)
    return out.astype(np.float32)


# revision 15
# speedup vs baseline: 8778.3399x; 1.8990x over previous

# Causal self-attention (B=8, T=1024, C=768, H=12) on 8 trn2 NeuronCores.
# Strategy: pure data parallelism — one batch element per core. Each core runs
# a fused QKV -> causal attention -> c_proj kernel written in Bass/Tile.
#
# Device-side layout choices (per core, T=1024, C=768, H=12, D=64):
#   - host pre-transposes x to xT [C, T] so every matmul contraction dim lands
#     on SBUF partitions; no on-device transposes anywhere.
#   - q,k are produced transposed (qT/kT [D,T] per head, 2 heads per 128-row
#     SBUF tile); v is produced in natural [T, C] layout.
#   - scoresT[tk, tq] = kT.T @ qT (contraction over D=64; two heads packed in
#     the PE array via row groups 0/64).
#   - softmax without max-subtraction (logits are bounded, exp is safe in f32);
#     exp on ScalarE over a 2-head [128, 1024] PSUM tile in one instruction;
#     causal masking applied post-exp on SBUF (triangle multiply + block
#     zeroing only on diagonal tiles).
#   - denominators come from an extra ones-matmul (column sums via PE),
#     replicated across 64 partitions so the reciprocal broadcast is free.
#   - PV matmul accumulates yT[c, t] directly in the layout c_proj needs.
#   - v-bias and proj-bias are folded on the host:
#       y = Pnorm @ (v~ + 1 b_v) = Pnorm @ v~ + 1 b_v
#       out = y @ w_proj + b_proj = (Pnorm @ v~) @ w_proj + (b_v @ w_proj + b_proj)
#     so the device never adds b_v; b_total is added once after c_proj.
#   - the 1/sqrt(D) scale and b_q are folded into w_attn's q columns (exact:
#     division by 8 is exact in fp32).
#   - all matmuls run as float32r (full PE rate at moving-dim >= 256).

import sys

sys.path.insert(0, "/opt/trn_rl_repo")

import numpy as np

import concourse.bass as bass
import concourse.bacc as bacc
import concourse.mybir as mybir
import concourse.tile as tile
from concourse.vector_clock import ScopedClock

B, T, C, H = 8, 1024, 768, 12
D = C // H  # 64
NCORES = 8
F32 = mybir.dt.float32
F32R = mybir.dt.float32r
BF16 = mybir.dt.bfloat16

# ---------------------------------------------------------------------------
# This walrus build accepts only one sync wait per CTRL instruction; the Tile
# tail drain aggregates one wait per logical processor. Split the excess waits
# onto dedicated NOPs ahead of the drain.
_MAX_WAITS = 1
_PATCHED = False


def _patch_tile_drain():
    global _PATCHED
    if _PATCHED:
        return
    _PATCHED = True

    def _drain_and_barrier(self, tick_clock, wait_clock):
        nc = self.nc
        bb = nc.cur_bb.bb
        idx_before = len(bb.instructions)
        drain_inst = nc.sync.drain()
        wait_clock.add_sem_waits(
            drain_inst.ins, ScopedClock({None: tick_clock.global_clock})
        )
        si = drain_inst.ins.sync_info
        if si is not None and si.on_wait and len(si.on_wait) > _MAX_WAITS:
            waits = list(si.on_wait)
            si.on_wait = waits[:_MAX_WAITS]
            extra = waits[_MAX_WAITS:]
            nops = []
            for i in range(0, len(extra), _MAX_WAITS):
                nop = nc.sync.nop(hint=f"drain_wait_spill_{i}", nofuse=True)
                nop.ins.sync_info = mybir.SyncInfo(
                    on_wait=extra[i : i + _MAX_WAITS], on_update=[]
                )
                nops.append(nop.ins)
            insts = bb.instructions
            tail = list(insts[idx_before:])
            assert tail[0] is drain_inst.ins
            del insts[idx_before:]
            for n in nops:
                insts.append(n)
            for t in tail:
                insts.append(t)
        nc.all_engine_barrier()
        popped = nc._tile_sem_poison_stack.pop()
        assert popped is self._sem_poison
        nc.clear_and_free_semaphores(list(self.sems.allocated().values()))
        nc.all_engine_barrier()

    tile.TileContext._drain_and_barrier = _drain_and_barrier


# ---------------------------------------------------------------------------


def build_program(loop_n=None, phases="lvqac"):
    _patch_tile_drain()
    nc = bacc.Bacc("TRN2", target_bir_lowering=False, debug=False)

    xT = nc.dram_tensor("xT", [C, T], F32R, kind="ExternalInput").ap()
    wA = nc.dram_tensor("w_attn", [C, 3 * C], F32R, kind="ExternalInput").ap()
    wP = nc.dram_tensor("w_proj", [C, C], F32R, kind="ExternalInput").ap()
    bqk = nc.dram_tensor("bqk", [128, 12], F32, kind="ExternalInput").ap()
    btot = nc.dram_tensor("btot", [1, C], F32, kind="ExternalInput").ap()
    ident = nc.dram_tensor("ident", [128, 128], BF16, kind="ExternalInput").ap()
    negtri = nc.dram_tensor("negtri", [128, 128], BF16, kind="ExternalInput").ap()
    ones64 = nc.dram_tensor("ones64", [128, D], BF16, kind="ExternalInput").ap()
    out = nc.dram_tensor("out", [T, C], F32, kind="ExternalOutput").ap()

    with tile.TileContext(nc) as tc:
        if loop_n is None:
            _emit_body(nc, tc, xT, wA, wP, bqk, btot, ident, negtri, ones64, out, phases)
        else:
            with tc.For_i(0, loop_n, 1):
                _emit_body(nc, tc, xT, wA, wP, bqk, btot, ident, negtri, ones64, out, phases)
    nc.compile()
    return nc


def _emit_body(nc, tc, xT, wA, wP, bqk, btot, ident, negtri, ones64, out, phases="lvqac"):
    EXP = mybir.ActivationFunctionType.Exp

    if True:
        with (
            tc.tile_pool(name="persist", bufs=1) as persist,
            tc.tile_pool(name="qk", bufs=6) as qk_pool,
            tc.tile_pool(name="ex", bufs=6) as ex_pool,
            tc.tile_pool(name="rb", bufs=3) as rb_pool,
            tc.tile_pool(name="ob", bufs=2) as ob_pool,
            tc.tile_pool(name="big_ps", bufs=3, space="PSUM") as big_ps,
            tc.tile_pool(name="acc_ps", bufs=2, space="PSUM") as acc_ps,
        ):
            # ---- persistent SBUF tensors -------------------------------
            xT_sb = []
            for c in range(6):
                t_ = persist.tile([128, T], F32R, tag=f"xT{c}")
                eng = (nc.gpsimd, nc.scalar, nc.sync)[c % 3]
                eng.dma_start(out=t_[:], in_=xT[128 * c : 128 * (c + 1), :])
                xT_sb.append(t_)
            wv_sb = []
            for c in range(6):
                t_ = persist.tile([128, C], F32R, tag=f"wv{c}")
                eng = (nc.scalar, nc.sync, nc.gpsimd)[c % 3]
                eng.dma_start(
                    out=t_[:], in_=wA[128 * c : 128 * (c + 1), 2 * C : 3 * C]
                )
                wv_sb.append(t_)
            wqk_sb = []
            for c in range(6):
                t_ = persist.tile([128, 2 * C], F32R, tag=f"wqk{c}", name=f"wqk{c}")
                eng = (nc.sync, nc.gpsimd, nc.scalar)[c % 3]
                eng.dma_start(
                    out=t_[:], in_=wA[128 * c : 128 * (c + 1), 0 : 2 * C]
                )
                wqk_sb.append(t_)
            wp_sb = []
            for c in range(6):
                t_ = persist.tile([128, C], F32R, tag=f"wp{c}")
                eng = (nc.gpsimd, nc.sync, nc.scalar)[c % 3]
                eng.dma_start(out=t_[:], in_=wP[128 * c : 128 * (c + 1), :])
                wp_sb.append(t_)
            bqk_sb = persist.tile([128, 12], F32, tag="bqk")
            nc.sync.dma_start(out=bqk_sb[:], in_=bqk[:, :])
            ident_sb = persist.tile([128, 128], BF16, tag="ident")
            nc.sync.dma_start(out=ident_sb[:], in_=ident[:, :])
            negtri_sb = persist.tile([128, 128], BF16, tag="negtri")
            nc.sync.dma_start(out=negtri_sb[:], in_=negtri[:, :])
            ones_sb = persist.tile([128, D], BF16, tag="ones")
            nc.sync.dma_start(out=ones_sb[:], in_=ones64[:, :])
            btot_sb = persist.tile([128, C], F32, tag="btot")
            btot_bcast = bass.AP(
                tensor=btot.tensor, offset=btot.offset, ap=[[0, 128], [1, C]]
            )
            nc.sync.dma_start(out=btot_sb[:], in_=btot_bcast)
            v_sb = [
                persist.tile([128, C], BF16, tag=f"v{t}", name=f"v{t}")
                for t in range(8)
            ]
            yT_sb = [
                persist.tile([128, T], F32R, tag=f"yT{p}", name=f"yT{p}")
                for p in range(6)
            ]

            # ---- V projection: v[t, c] natural layout ------------------
            for t in range(8 if "v" in phases else 0):
                vps = big_ps.tile([128, C], F32, tag="scores")
                for c in range(6):
                    for n0, nw in ((0, 512), (512, 256)):
                        nc.tensor.matmul(
                            vps[:, n0 : n0 + nw],
                            lhsT=xT_sb[c][:, 128 * t : 128 * (t + 1)],
                            rhs=wv_sb[c][:, n0 : n0 + nw],
                            start=(c == 0),
                            stop=(c == 5),
                        )
                nc.vector.tensor_copy(v_sb[t][:], vps[:, :])

            # ---- per head-pair: q/k projection + attention -------------
            for p in range(6 if "q" in phases else 0):
                qk_tiles = {}
                for kind, jt in (("q", p), ("k", 6 + p)):
                    qk = qk_pool.tile([128, T], F32R, tag=f"qk_{kind}")
                    for th in range(2):
                        ps = big_ps.tile([128, 512], F32, tag="scores")
                        for c in range(6):
                            nc.tensor.matmul(
                                ps[:, :],
                                lhsT=wqk_sb[c][:, 128 * jt : 128 * (jt + 1)],
                                rhs=xT_sb[c][:, 512 * th : 512 * (th + 1)],
                                start=(c == 0),
                                stop=(c == 5),
                            )
                        nc.vector.tensor_scalar_add(
                            qk[:, 512 * th : 512 * (th + 1)],
                            ps[:, :],
                            bqk_sb[:, jt : jt + 1],
                        )
                    qk_tiles[kind] = qk
                qT, kT = qk_tiles["q"], qk_tiles["k"]

                for i in range(2 if "a" in phases else 0):
                    pv = acc_ps.tile([128, 512], F32, tag="pv", bufs=1)
                    sm = acc_ps.tile([128, 512], F32, tag="sums", bufs=1)
                    njt = 4 * i + 4  # causal tk tiles for this tq half
                    for j in range(njt):
                        r = j - 4 * i  # >=0 on diagonal tiles
                        # fully-masked tq-col prefix of this tile (in 128-blocks)
                        off = 128 * r if r > 0 else 0
                        # scores: fp32r needs moving dim >= 256 for full rate
                        s_off = min(off, 256)
                        sc = big_ps.tile([128, 1024], F32, tag="scores")
                        sc3 = sc[:].rearrange("p (h c) -> p h c", h=2)
                        for h in range(2):
                            nc.tensor.matmul(
                                sc3[:, h, s_off:512],
                                lhsT=kT[64 * h : 64 * (h + 1), 128 * j : 128 * (j + 1)],
                                rhs=qT[64 * h : 64 * (h + 1), 512 * i + s_off : 512 * (i + 1)],
                                tile_position=(64 * h, 0),
                            )
                        if r >= 0:
                            for h in range(2):
                                nc.tensor.matmul(
                                    sc3[:, h, 128 * r : 128 * (r + 1)],
                                    lhsT=ident_sb[:],
                                    rhs=negtri_sb[:],
                                    start=False,
                                    stop=True,
                                    skip_group_check=True,
                                )
                        ex = ex_pool.tile([128, 1024], BF16, tag="ex")
                        ex3 = ex[:].rearrange("p (h c) -> p h c", h=2)
                        nc.scalar.activation(
                            ex3[:, :, off:512], sc3[:, :, off:512], EXP
                        )
                        first, last = (j == 0), (j == njt - 1)
                        for h in range(2):
                            hd = 2 * p + h
                            nc.tensor.matmul(
                                pv[64 * h : 64 * (h + 1), off:512],
                                lhsT=v_sb[j][:, D * hd : D * (hd + 1)],
                                rhs=ex3[:, h, off:512],
                                tile_position=(0, 64 * h),
                                start=first,
                                stop=last,
                            )
                            nc.tensor.matmul(
                                sm[64 * h : 64 * (h + 1), off:512],
                                lhsT=ones_sb[:],
                                rhs=ex3[:, h, off:512],
                                tile_position=(0, 64 * h),
                                start=first,
                                stop=last,
                            )
                    rb = rb_pool.tile([128, 512], F32, tag="rb")
                    nc.vector.reciprocal_approx_fast(rb[:], sm[:, :])
                    nc.vector.tensor_mul(
                        yT_sb[p][:, 512 * i : 512 * (i + 1)], pv[:, :], rb[:]
                    )

            # ---- c_proj ------------------------------------------------
            for t in range(8 if "c" in phases else 0):
                cps = big_ps.tile([128, C], F32, tag="scores")
                for c in range(6):
                    for n0, nw in ((0, 512), (512, 256)):
                        nc.tensor.matmul(
                            cps[:, n0 : n0 + nw],
                            lhsT=yT_sb[c][:, 128 * t : 128 * (t + 1)],
                            rhs=wp_sb[c][:, n0 : n0 + nw],
                            start=(c == 0),
                            stop=(c == 5),
                        )
                ob = ob_pool.tile([128, C], F32, tag="ob")
                nc.vector.tensor_add(ob[:], cps[:, :], btot_sb[:])
                (nc.gpsimd, nc.scalar)[t % 2].dma_start(out=out[128 * t : 128 * (t + 1), :], in_=ob[:])


_NC = None


def _get_nc():
    global _NC
    if _NC is None:
        _NC = build_program()
    return _NC


def round_fp32r(a):
    """Round fp32 to fp32r (e8m11): round-to-nearest-even at mantissa bit 12.
    Matches walrus's fp32_to_fp32r (downconv to e8m11, stored in the high 20
    bits, i.e. fp32 with the low 12 mantissa bits zero)."""
    u = np.ascontiguousarray(a, dtype=np.float32).view(np.uint32)
    r = (u + np.uint32(0x7FF) + ((u >> np.uint32(12)) & np.uint32(1))) & np.uint32(
        0xFFFFF000
    )
    return r.view(np.float32)


def make_inputs(x, w_attn, b_attn, w_proj, b_proj):
    """Host-side prep: fold scales/biases, transpose x, build constants."""
    x = np.asarray(x, dtype=np.float32)
    w_attn = np.asarray(w_attn, dtype=np.float32)
    b_attn = np.asarray(b_attn, dtype=np.float32)
    w_proj = np.asarray(w_proj, dtype=np.float32)
    b_proj = np.asarray(b_proj, dtype=np.float32)

    wA = w_attn.copy()
    wA[:, :C] *= 0.125  # fold 1/sqrt(D)=1/8 into q columns (exact in fp32)
    bq = b_attn[:C] * 0.125
    bk = b_attn[C : 2 * C]
    bv = b_attn[2 * C :]
    # bqk[p, j] = bias for feature j*128+p, j in 0..11 (q tiles then k tiles)
    bqk = np.concatenate([bq, bk]).reshape(12, 128).T.copy()
    btot = (b_proj.astype(np.float64) + bv.astype(np.float64) @ w_proj.astype(np.float64)).astype(
        np.float32
    )[None, :]
    import ml_dtypes
    ident = np.eye(128, dtype=np.float32).astype(ml_dtypes.bfloat16)
    # negtri[tk, tq] = -1e30 where tq < tk (masked), else 0
    negtri = np.where(
        np.arange(128)[None, :] < np.arange(128)[:, None], -1e30, 0.0
    ).astype(np.float32).astype(ml_dtypes.bfloat16)
    ones64 = np.ones((128, D), dtype=ml_dtypes.bfloat16)

    shared = {
        "w_attn": round_fp32r(wA),
        "w_proj": round_fp32r(w_proj),
        "bqk": np.ascontiguousarray(bqk),
        "btot": np.ascontiguousarray(btot),
        "ident": np.ascontiguousarray(ident),
        "negtri": np.ascontiguousarray(negtri),
        "ones64": ones64,
    }
    in_maps = []
    for b in range(B):
        m = dict(shared)
        m["xT"] = round_fp32r(x[b].T)
        in_maps.append(m)
    return in_maps


def kernel(x, w_attn, b_attn, w_proj, b_proj):
    import time

    from concourse.bass_utils import run_bass_kernel_spmd

    nc = _get_nc()
    in_maps = make_inputs(x, w_attn, b_attn, w_proj, b_proj)
    res = None
    for attempt in range(4):
        try:
            res = run_bass_kernel_spmd(nc, in_maps, list(range(NCORES)))
            break
        except Exception:
            if attempt == 3:
                raise
            time.sleep(30)  # give a wedged NeuronCore time to recover
    out = np.stack([res.results[b]["out"] for b in range(B)], axis=0)
    return out.astype(np.float32)
